# revision 1
# baseline (speedup 1.0000x reference)
"""RNNT joint log_softmax kernel for Trainium2 (Bass/Tile), 8-core SPMD.

out[b,t,u,v] = log_softmax(f[b,t,v] + g[b,u,v], axis=v)

Sharding: 8 shards over (b, t-half): core i handles b=i//2, t in [128*(i%2), ...).
Per-core trick: exp(f+g) = exp(f)*exp(g), so every (t,u) logsumexp comes from one
small matmul S = exp(g) @ exp(f)^T contracted over v, then lse = ln(S).
Main loop per t: PE rank-1 broadcast of f_t (bf16 hi/lo split, exact to ~2^-16)
into PSUM, ACT adds -lse (per-partition bias), DVE adds G, DMA out.
"""

import numpy as np

B, T, U, V = 4, 256, 128, 1024
TSH = 128  # t-shard per core
NCORES = 8

_nc_cache = {}


def _build(tag="main"):
    if tag in _nc_cache:
        return _nc_cache[tag]
    from contextlib import ExitStack

    import concourse.bacc as bacc
    import concourse.tile as tile
    from concourse import mybir

    f32 = mybir.dt.float32
    bf16 = mybir.dt.bfloat16
    AF = mybir.ActivationFunctionType

    nc = bacc.Bacc("TRN2", debug=False, num_devices=NCORES)
    f_d = nc.dram_tensor("f_sh", [TSH, V], f32, kind="ExternalInput").ap()
    g_d = nc.dram_tensor("g_sh", [U, V], f32, kind="ExternalInput").ap()
    eye_d = nc.dram_tensor("eye", [128, 128], f32, kind="ExternalInput").ap()
    out_d = nc.dram_tensor("out_sh", [TSH, U, V], f32, kind="ExternalOutput").ap()

    with tile.TileContext(nc) as tc, ExitStack() as ctx:
        const_pool = ctx.enter_context(tc.tile_pool(name="const", bufs=1))
        out_pool = ctx.enter_context(tc.tile_pool(name="out", bufs=4))

        F = const_pool.tile([128, V], f32)
        G = const_pool.tile([128, V], f32)
        eye = const_pool.tile([128, 128], f32)
        nc.sync.dma_start(F[:], f_d[:])
        nc.sync.dma_start(G[:], g_d[:])
        nc.sync.dma_start(eye[:], eye_d[:])

        eye_bf = const_pool.tile([128, 128], bf16)
        nc.vector.tensor_copy(eye_bf[:], eye[:])

        # --- preamble: transposed exp tiles + S matmul -> -lse[u, t] ---
        EfT = const_pool.tile([128, V], f32)  # chunk c at [:, 128c:128c+128]
        EgT = const_pool.tile([128, V], f32)
        lseT = const_pool.tile([128, 128], f32)
        neg_lseT = const_pool.tile([128, 128], f32)
        with tc.tile_pool(name="psum_pre", bufs=2, space="PSUM") as pre_psum, \
             tc.tile_pool(name="psum_s", bufs=1, space="PSUM") as s_pool:
            for src, dst in ((F, EfT), (G, EgT)):
                for c in range(8):
                    tp = pre_psum.tile([128, 128], f32, tag="tp")
                    nc.tensor.transpose(tp[:], src[:, 128 * c:128 * (c + 1)], eye[:])
                    nc.scalar.activation(dst[:, 128 * c:128 * (c + 1)], tp[:], AF.Exp)
            s_ps = s_pool.tile([128, 128], f32)
            for c in range(8):
                nc.tensor.matmul(
                    s_ps[:],
                    EgT[:, 128 * c:128 * (c + 1)],
                    EfT[:, 128 * c:128 * (c + 1)],
                    start=(c == 0),
                    stop=(c == 7),
                )
            nc.scalar.activation(lseT[:], s_ps[:], AF.Ln)
        nc.scalar.mul(neg_lseT[:], lseT[:], -1.0)

        # --- bf16 hi/lo split of F for exact-ish PE broadcast ---
        F_hi = const_pool.tile([128, V], bf16)
        F_hi32 = const_pool.tile([128, V], f32)
        F_lo32 = const_pool.tile([128, V], f32)
        F_lo = const_pool.tile([128, V], bf16)
        nc.vector.tensor_copy(F_hi[:], F[:])
        nc.vector.tensor_copy(F_hi32[:], F_hi[:])
        nc.vector.tensor_sub(F_lo32[:], F[:], F_hi32[:])
        nc.vector.tensor_copy(F_lo[:], F_lo32[:])

        # --- main loop over t, grouped GT t's per output DMA (2MB writes) ---
        GT = 4
        with tc.tile_pool(name="psum_b", bufs=4, space="PSUM") as psum_b:
            for tg in range(TSH // GT):
                stage = out_pool.tile([128, GT, V], f32)
                for j in range(GT):
                    t = tg * GT + j
                    pb = psum_b.tile([128, V], f32)
                    # one-hot column t of eye as stationary operand: selects
                    # row t of F_hi/F_lo (broadcast over all output partitions)
                    onehot = eye_bf[:, t:t + 1].broadcast_to([128, 128])
                    for c2 in range(2):
                        sl = slice(512 * c2, 512 * (c2 + 1))
                        nc.tensor.matmul(
                            pb[:, sl], onehot, F_hi[:, sl],
                            start=True, stop=False,
                        )
                        nc.tensor.matmul(
                            pb[:, sl], onehot, F_lo[:, sl],
                            start=False, stop=True,
                        )
                    # pb += -lse[:, t]  (per-partition scalar bias on ACT)
                    nc.scalar.activation(
                        pb[:], pb[:], AF.Identity, bias=neg_lseT[:, t:t + 1]
                    )
                    nc.vector.tensor_add(stage[:, j, :], G[:], pb[:])
                nc.sync.dma_start(
                    out_d[tg * GT:(tg + 1) * GT].rearrange("t u v -> u t v"),
                    stage[:],
                )

    nc.compile()
    _nc_cache[tag] = nc
    return nc


def _in_maps(f, g):
    eye = np.eye(128, dtype=np.float32)
    maps = []
    for i in range(NCORES):
        b, h = divmod(i, 2)
        maps.append({
            "f_sh": np.ascontiguousarray(f[b, h * TSH:(h + 1) * TSH]),
            "g_sh": np.ascontiguousarray(g[b]),
            "eye": eye,
        })
    return maps


def _gather(results):
    out = np.empty((B, T, U, V), np.float32)
    for i in range(NCORES):
        b, h = divmod(i, 2)
        out[b, h * TSH:(h + 1) * TSH] = results[i]["out_sh"]
    return out


def kernel(**inputs):
    from concourse.bass_utils import run_bass_kernel_spmd

    f = np.asarray(inputs["f"], np.float32)
    g = np.asarray(inputs["g"], np.float32)
    nc = _build()
    res = run_bass_kernel_spmd(nc, _in_maps(f, g), core_ids=list(range(NCORES)))
    return _gather(res.results)



# revision 30
# speedup vs baseline: 1.8350x; 1.8350x over previous
"""RNNT joint log_softmax kernel for Trainium2 (Bass/Tile), 8-core SPMD.

out[b,t,u,v] = log_softmax(f[b,t,v] + g[b,u,v], axis=v)

Sharding: 8 shards over (b, t-half): core i handles b=i//2, t in [128*(i%2), ...).
lse trick: exp(f+g) = exp(f)*exp(g), so lse[t,u] = ln(exp(f) @ exp(g)^T).

Main loop (partitions = t, loop over u): moving tile B = [F rows 0:123 |
4 g-rows | ones row], stationary W_u = diag(123) + g-select row + a row
holding -lse[:,u] (written per-u by Pool from SBUF), so ONE matmul pair per u
computes the ENTIRE output F[t,v] + g_u[v] - lse[t,u] for 123 t-rows in PSUM.
The epilogue is then a pure PSUM->fp16 downcast with no per-u scalar, so it
merges across u-pairs and splits by v-columns across DVE (tensor_copy) and
ACT (activation Copy); GPSIMD cannot touch PSUM, so Pool instead feeds the
W-row updates, B F-rows and negations (all SBUF-side). Inputs land as f16
(host-converted) and output is written fp16 (rel err ~5e-3 << 2e-2 gate),
halving HBM traffic; host upcasts. All ACT functions (Exp/Ln/Copy) are
steered into the one table set containing them so a single LoadActFuncSet
runs off the critical path. Leftover t-rows 123..127 use a one-hot broadcast
path (lse via a K=1 ones matmul), interleaved mid-loop.
"""

import numpy as np

B, T, U, V = 4, 256, 128, 1024
TSH = 128          # t-shard per core
NCORES = 8
TMAIN = 123        # t-rows handled by the B-tile trick per matmul
NG = 4             # g-rows resident in B / u's per group
NGRP = U // NG     # 32 groups
NB = 4             # B-tile ring
NW = 8             # W-stationary ring
XD = 510           # epilogue cols on DVE; rest on ACT

_nc_cache = {}


def _build(tag="main"):
    if tag in _nc_cache:
        return _nc_cache[tag]
    from contextlib import ExitStack

    import concourse.bacc as bacc
    import concourse.tile as tile
    from concourse import mybir

    f32 = mybir.dt.float32
    f16 = mybir.dt.float16
    AF = mybir.ActivationFunctionType

    # Steer Exp/Ln/Identity/Copy into the single table set holding them all
    # ("natural_log_exp_and_others") so only one LoadActFuncSet is emitted.
    # Set indices/IDs are unchanged and the chosen set genuinely contains
    # these functions, so the emitted BIR stays valid for walrus.
    _orig_tables = bacc.get_activation_tables

    def _steered_tables(arch):
        out = {}
        for name, funcs in _orig_tables(arch).items():
            funcs = set(funcs)
            if name != "natural_log_exp_and_others":
                funcs.discard(AF.Exp)
                funcs.discard(AF.Ln)
                funcs.discard(AF.Identity)
                funcs.discard(AF.Copy)
            out[name] = funcs
        return out

    nc = bacc.Bacc("TRN2", debug=False, num_devices=NCORES)
    f_d = nc.dram_tensor("f_sh", [TSH, V], f16, kind="ExternalInput").ap()
    g_d = nc.dram_tensor("g_sh", [U, V], f16, kind="ExternalInput").ap()
    eye_d = nc.dram_tensor("eye16", [128, 128], f16, kind="ExternalInput").ap()
    w8_d = nc.dram_tensor("w8", [128, NG * 128], f16, kind="ExternalInput").ap()
    out_d = nc.dram_tensor("out_sh", [TSH, U, V], f16, kind="ExternalOutput").ap()

    with tile.TileContext(nc) as tc, ExitStack() as ctx:
        const_pool = ctx.enter_context(tc.tile_pool(name="const", bufs=1))

        F16 = const_pool.tile([128, V], f16)
        G16 = const_pool.tile([128, V], f16)
        eye16 = const_pool.tile([128, 128], f16)
        W8 = const_pool.tile([128, NG * 128], f16)
        scr = const_pool.tile([128, 128], f16)
        ones1 = const_pool.tile([1, V], f16)
        nc.vector.memset(scr[:], 0.0)
        nc.vector.memset(ones1[:], 1.0)

        nc.sync.dma_start(eye16[:], eye_d[:])
        # halves so transposes of the first chunks start earlier
        nc.sync.dma_start(F16[:, :512], f_d[:, :512])
        nc.sync.dma_start(F16[:, 512:], f_d[:, 512:])
        nc.sync.dma_start(W8[:], w8_d[:])
        nc.sync.dma_start(G16[:, :512], g_d[:, :512])
        nc.sync.dma_start(G16[:, 512:], g_d[:, 512:])

        # --- B ring layout (partition-window rule: engine APs must start at
        # 0/32/64/96): rows 0:96 = F[0:96], row 96 = ones, rows 97:101 = the
        # group's 4 g-rows, rows 101:128 = F[96:123]. g-seeds and the F tail
        # go via DMA (arbitrary partitions allowed there).
        Bt = [const_pool.tile([128, V], f16, name=f"Bt{q}") for q in range(NB)]
        for q in range(NB):
            nc.vector.tensor_copy(Bt[q][0:96, :], F16[0:96, :])
            nc.vector.memset(Bt[q][96:97, :], 1.0)
            nc.gpsimd.dma_start(Bt[q][97:101, :], g_d[NG * q:NG * (q + 1), :])
            nc.gpsimd.dma_start(Bt[q][101:128, :], f_d[96:TMAIN, :])

        # --- W ring: static part from host (diag + g-select); row 127 gets
        # -lseT[u] written per-u by Pool.
        Wb = [const_pool.tile([128, 128], f16, name=f"Wb{q}") for q in range(NW)]
        for q in range(NW):
            nc.gpsimd.tensor_copy(
                Wb[q][:, :], W8[:, 128 * (q % NG):128 * (q % NG) + 128])

        # --- preamble: lse in both orientations via exp-transpose-matmul ---
        EfT = const_pool.tile([128, V], f16)   # col block c: [v-chunk, t]
        EgT = const_pool.tile([128, V], f16)
        neg_lse16 = const_pool.tile([128, 128], f16)    # [t, u]
        neg_lseT16 = const_pool.tile([128, 128], f16)   # [u, t]
        with tc.tile_pool(name="psum_pre", bufs=2, space="PSUM") as pre_psum, \
             tc.tile_pool(name="psum_s", bufs=1, space="PSUM") as s_pool:
            # PE p-state warmup while input DMAs land (results unused);
            # scr needs no DMA, so PE ramps from t~0 into the real transposes
            warm = pre_psum.tile([128, 128], f16, tag="warm")
            for _ in range(24):
                nc.tensor.transpose(warm[:], scr[:], scr[:])
            for src, dst in ((F16, EfT), (G16, EgT)):
                for h in range(2):          # two 512-col halves, 4 chunks each
                    tp = pre_psum.tile([128, 512], f16, tag="tp")
                    for i in range(4):
                        c = 4 * h + i
                        nc.tensor.transpose(
                            tp[:, 128 * i:128 * (i + 1)],
                            src[:, 128 * c:128 * (c + 1)], eye16[:])
                    nc.scalar.activation(
                        dst[:, 512 * h:512 * (h + 1)], tp[:], AF.Exp)
            lse_t = const_pool.tile([128, 128], f32)
            lse_u = const_pool.tile([128, 128], f32)
            s_ps = s_pool.tile([128, 128], f32, tag="s")
            s_psT = s_pool.tile([128, 128], f32, tag="sT")
            for c in range(8):
                sl = slice(128 * c, 128 * (c + 1))
                nc.tensor.matmul(s_psT[:], EgT[:, sl], EfT[:, sl],
                                 start=(c == 0), stop=(c == 7))
            for c in range(8):
                sl = slice(128 * c, 128 * (c + 1))
                nc.tensor.matmul(s_ps[:], EfT[:, sl], EgT[:, sl],
                                 start=(c == 0), stop=(c == 7))
            nc.scalar.activation(lse_u[:], s_psT[:], AF.Ln)
            nc.gpsimd.tensor_scalar_mul(neg_lseT16[:], lse_u[:], -1.0)
            nc.scalar.activation(lse_t[:], s_ps[:], AF.Ln)
            nc.gpsimd.tensor_scalar_mul(neg_lse16[:], lse_t[:], -1.0)

        # flatten -lseT into partition 0 (DMA may cross partitions freely)
        # so Pool can read any u's row from an aligned window; same for the
        # leftover rows of -lse.
        L0 = const_pool.tile([1, 128 * 128], f16)
        nc.sync.dma_start(L0[0:1, :], neg_lseT16[:, :])
        L0T = const_pool.tile([1, (TSH - TMAIN) * 128], f16)
        nc.sync.dma_start(L0T[0:1, :], neg_lse16[TMAIN:TSH, :])
        # seed W lse rows (at partition 96) for u = 0..NW-1
        for u in range(NW):
            nc.gpsimd.tensor_copy(
                Wb[u][96:97, :TMAIN], L0[0:1, 128 * u:128 * u + TMAIN])

        # --- main loop: NGRP groups x NG u's; leftover t-rows interleaved ---
        out_pool = ctx.enter_context(tc.tile_pool(name="out", bufs=6))
        lo_pool = ctx.enter_context(tc.tile_pool(name="lo", bufs=2))
        nlo = TSH - TMAIN
        lo_every = NGRP // nlo
        with tc.tile_pool(name="psum_b", bufs=4, space="PSUM") as psum_b:
            for m in range(NGRP):
                Bb = Bt[m % NB]
                # separate stage tiles per engine: no shared writer tiles,
                # so no cross-engine serialization through the tracker
                stageD = out_pool.tile([TMAIN, 2, V], f16, tag="sD")
                stageA = out_pool.tile([TMAIN, 2, V], f16, tag="sA")
                for j in range(NG):
                    u = NG * m + j
                    Wu = Wb[u % NW]
                    pb = psum_b.tile([128, V], f32, tag="pb")
                    for c2 in range(2):
                        bsl = slice(512 * c2, 512 * (c2 + 1))
                        nc.tensor.matmul(
                            pb[:TMAIN, bsl], Wu[:, :TMAIN],
                            Bb[:, bsl], start=True, stop=True)
                    # refresh this W tile's lse row for u+NW (Pool,
                    # SBUF-only; WAR on the matmuls above)
                    if u + NW < U:
                        un = u + NW
                        nc.gpsimd.tensor_copy(
                            Wu[96:97, :TMAIN],
                            L0[0:1, 128 * un:128 * un + TMAIN])
                    # epilogue: pure PSUM->f16 downcast; DVE takes even u's,
                    # ACT odd u's — each with its own psum and stage tile
                    if j % 2 == 0:
                        nc.vector.tensor_copy(
                            stageD[:, j // 2, :], pb[:TMAIN, :])
                    else:
                        nc.scalar.activation(
                            stageA[:, j // 2, :], pb[:TMAIN, :], AF.Copy)
                # refresh this B tile's g-rows for group m+NB
                if m + NB < NGRP:
                    nc.sync.dma_start(
                        Bt[m % NB][97:101, :],
                        G16[NG * (m + NB):NG * (m + NB + 1), :],
                    )
                u0 = NG * m
                nc.sync.dma_start(
                    out_d[0:TMAIN, u0:u0 + NG:2, :], stageD[:])
                nc.sync.dma_start(
                    out_d[0:TMAIN, u0 + 1:u0 + NG:2, :], stageA[:])

                # one leftover t-row (one-hot f_t broadcast, u-partitions;
                # lse added via a K=1 ones-row matmul)
                if m % lo_every == lo_every // 2 - 1:
                    t = TMAIN + m // lo_every
                    lse_row = L0T[0:1, 128 * (t - TMAIN):128 * (t - TMAIN + 1)]
                    pb2 = psum_b.tile([128, V], f32, tag="pb")
                    onehot = eye16[:, t:t + 1].broadcast_to([128, 128])
                    for c2 in range(2):
                        sl = slice(512 * c2, 512 * (c2 + 1))
                        nc.tensor.matmul(pb2[:, sl], onehot, F16[:, sl],
                                         start=True, stop=False)
                        nc.tensor.matmul(pb2[:, sl], eye16[:], G16[:, sl],
                                         start=False, stop=False)
                        nc.tensor.matmul(pb2[:, sl], lse_row,
                                         ones1[:, sl], start=False, stop=True)
                    stage2 = lo_pool.tile([128, V], f16)
                    nc.vector.tensor_copy(stage2[:, 0:XD], pb2[:, 0:XD])
                    nc.scalar.activation(stage2[:, XD:V], pb2[:, XD:V],
                                         AF.Copy)
                    nc.sync.dma_start(out_d[t, :, :], stage2[:])

    bacc.get_activation_tables = _steered_tables
    try:
        nc.compile()
    finally:
        bacc.get_activation_tables = _orig_tables
    _nc_cache[tag] = nc
    return nc


def _consts():
    eye16 = np.eye(128, dtype=np.float16)
    # B-row map: rows 0:96 = F[0:96], row 96 = ones (lse row in W),
    # rows 97:101 = g-rows, rows 101:128 = F[96:123]
    w8 = np.zeros((128, NG * 128), dtype=np.float16)
    for j in range(NG):
        blk = w8[:, 128 * j:128 * (j + 1)]
        for k in range(96):
            blk[k, k] = 1.0
        for k in range(101, 128):
            blk[k, k - 5] = 1.0
        for t in range(TMAIN):
            blk[97 + j, t] = 1.0
    return eye16, w8


def _in_maps(f, g):
    eye16, w8 = _consts()
    f16 = f.astype(np.float16)
    g16 = g.astype(np.float16)
    maps = []
    for i in range(NCORES):
        b, h = divmod(i, 2)
        maps.append({
            "f_sh": np.ascontiguousarray(f16[b, h * TSH:(h + 1) * TSH]),
            "g_sh": np.ascontiguousarray(g16[b]),
            "eye16": eye16,
            "w8": w8,
        })
    return maps


def _gather(results):
    out = np.empty((B, T, U, V), np.float32)
    for i in range(NCORES):
        b, h = divmod(i, 2)
        out[b, h * TSH:(h + 1) * TSH] = np.asarray(
            results[i]["out_sh"], dtype=np.float32)
    return out


def kernel(**inputs):
    from concourse.bass_utils import run_bass_kernel_spmd

    f = np.asarray(inputs["f"], np.float32)
    g = np.asarray(inputs["g"], np.float32)
    nc = _build()
    res = run_bass_kernel_spmd(nc, _in_maps(f, g), core_ids=list(range(NCORES)))
    return _gather(res.results)


# revision 33
# speedup vs baseline: 1.9155x; 1.0438x over previous
"""RNNT joint log_softmax kernel for Trainium2 (Bass/Tile), 8-core SPMD.

out[b,t,u,v] = log_softmax(f[b,t,v] + g[b,u,v], axis=v)

Sharding: 8 shards over (b, t-half): core i handles b=i//2, t in [128*(i%2), ...).
lse trick: exp(f+g) = exp(f)*exp(g), so lse[t,u] = ln(exp(f) @ exp(g)^T).

Main loop (partitions = t, loop over u): moving tile B = [F rows 0:123 |
4 g-rows | ones row], stationary W_u = diag(123) + g-select row + a row
holding -lse[:,u] (written per-u by Pool from SBUF), so ONE matmul pair per u
computes the ENTIRE output F[t,v] + g_u[v] - lse[t,u] for 123 t-rows in PSUM.
The epilogue is then a pure PSUM->fp16 downcast with no per-u scalar, so it
merges across u-pairs and splits by v-columns across DVE (tensor_copy) and
ACT (activation Copy); GPSIMD cannot touch PSUM, so Pool instead feeds the
W-row updates, B F-rows and negations (all SBUF-side). Inputs land as f16
(host-converted) and output is written fp16 (rel err ~5e-3 << 2e-2 gate),
halving HBM traffic; host upcasts. All ACT functions (Exp/Ln/Copy) are
steered into the one table set containing them so a single LoadActFuncSet
runs off the critical path. Leftover t-rows 123..127 use a one-hot broadcast
path (lse via a K=1 ones matmul), interleaved mid-loop.
"""

import numpy as np

B, T, U, V = 4, 256, 128, 1024
TSH = 128          # t-shard per core
NCORES = 8
TMAIN = 123        # t-rows handled by the B-tile trick per matmul
NG = 4             # g-rows resident in B / u's per group
NGRP = U // NG     # 32 groups
NB = 4             # B-tile ring
NW = 8             # W-stationary ring
XD = 510           # epilogue cols on DVE; rest on ACT

_nc_cache = {}


def _build(tag="main"):
    if tag in _nc_cache:
        return _nc_cache[tag]
    from contextlib import ExitStack

    import concourse.bacc as bacc
    import concourse.tile as tile
    from concourse import mybir

    f32 = mybir.dt.float32
    f16 = mybir.dt.float16
    AF = mybir.ActivationFunctionType

    # Steer Exp/Ln/Identity/Copy into the single table set holding them all
    # ("natural_log_exp_and_others") so only one LoadActFuncSet is emitted.
    # Set indices/IDs are unchanged and the chosen set genuinely contains
    # these functions, so the emitted BIR stays valid for walrus.
    _orig_tables = bacc.get_activation_tables

    def _steered_tables(arch):
        out = {}
        for name, funcs in _orig_tables(arch).items():
            funcs = set(funcs)
            if name != "natural_log_exp_and_others":
                funcs.discard(AF.Exp)
                funcs.discard(AF.Ln)
                funcs.discard(AF.Identity)
                funcs.discard(AF.Copy)
            out[name] = funcs
        return out

    nc = bacc.Bacc("TRN2", debug=False, num_devices=NCORES)
    f_d = nc.dram_tensor("f_sh", [TSH, V], f16, kind="ExternalInput").ap()
    g_d = nc.dram_tensor("g_sh", [U, V], f16, kind="ExternalInput").ap()
    eye_d = nc.dram_tensor("eye16", [128, 128], f16, kind="ExternalInput").ap()
    w8_d = nc.dram_tensor("w8", [128, NG * 128], f16, kind="ExternalInput").ap()
    out_d = nc.dram_tensor("out_sh", [TSH, U, V], f16, kind="ExternalOutput").ap()

    with tile.TileContext(nc) as tc, ExitStack() as ctx:
        const_pool = ctx.enter_context(tc.tile_pool(name="const", bufs=1))

        F16 = const_pool.tile([128, V], f16)
        G16 = const_pool.tile([128, V], f16)
        eye16 = const_pool.tile([128, 128], f16)
        W8 = const_pool.tile([128, NG * 128], f16)
        scr = const_pool.tile([128, 128], f16)
        ones1 = const_pool.tile([1, V], f16)
        nc.vector.memset(scr[:], 0.0)
        nc.vector.memset(ones1[:], 1.0)

        nc.sync.dma_start(eye16[:], eye_d[:])
        # halves so transposes of the first chunks start earlier
        nc.sync.dma_start(F16[:, :512], f_d[:, :512])
        nc.sync.dma_start(F16[:, 512:], f_d[:, 512:])
        nc.sync.dma_start(W8[:], w8_d[:])
        nc.sync.dma_start(G16[:, :512], g_d[:, :512])
        nc.sync.dma_start(G16[:, 512:], g_d[:, 512:])

        # --- B ring layout (partition-window rule: engine APs must start at
        # 0/32/64/96): rows 0:96 = F[0:96], row 96 = ones, rows 97:101 = the
        # group's 4 g-rows, rows 101:128 = F[96:123]. g-seeds and the F tail
        # go via DMA (arbitrary partitions allowed there).
        Bt = [const_pool.tile([128, V], f16, name=f"Bt{q}") for q in range(NB)]
        for q in range(NB):
            nc.vector.tensor_copy(Bt[q][0:96, :], F16[0:96, :])
            nc.vector.memset(Bt[q][96:97, :], 1.0)
            nc.gpsimd.dma_start(Bt[q][97:101, :], g_d[NG * q:NG * (q + 1), :])
            nc.gpsimd.dma_start(Bt[q][101:128, :], f_d[96:TMAIN, :])

        # --- W ring: static part from host (diag + g-select); row 127 gets
        # -lseT[u] written per-u by Pool.
        Wb = [const_pool.tile([128, 128], f16, name=f"Wb{q}") for q in range(NW)]
        for q in range(NW):
            nc.gpsimd.tensor_copy(
                Wb[q][:, :], W8[:, 128 * (q % NG):128 * (q % NG) + 128])

        # --- preamble: lse in both orientations via exp-transpose-matmul ---
        EfT = const_pool.tile([128, V], f16)   # col block c: [v-chunk, t]
        EgT = const_pool.tile([128, V], f16)
        neg_lse16 = const_pool.tile([128, 128], f16)    # [t, u]
        neg_lseT16 = const_pool.tile([128, 128], f16)   # [u, t]
        with tc.tile_pool(name="psum_pre", bufs=2, space="PSUM") as pre_psum, \
             tc.tile_pool(name="psum_s", bufs=1, space="PSUM") as s_pool:
            # PE p-state warmup while input DMAs land (results unused);
            # scr needs no DMA, so PE ramps from t~0 into the real transposes
            warm = pre_psum.tile([128, 128], f16, tag="warm")
            for _ in range(24):
                nc.tensor.transpose(warm[:], scr[:], scr[:])
            for src, dst in ((F16, EfT), (G16, EgT)):
                for h in range(2):          # two 512-col halves, 4 chunks each
                    tp = pre_psum.tile([128, 512], f16, tag="tp")
                    for i in range(4):
                        c = 4 * h + i
                        nc.tensor.transpose(
                            tp[:, 128 * i:128 * (i + 1)],
                            src[:, 128 * c:128 * (c + 1)], eye16[:])
                    nc.scalar.activation(
                        dst[:, 512 * h:512 * (h + 1)], tp[:], AF.Exp)
            lse_t32 = const_pool.tile([128, 128], f32)
            neg_lse32 = const_pool.tile([128, 128], f32)
            lse_u = const_pool.tile([128, 128], f32)
            s_ps = s_pool.tile([128, 128], f32, tag="s")
            s_psT = s_pool.tile([128, 128], f32, tag="sT")
            # t-orientation first: it unblocks the hybrid scalar epilogue of
            # the first two groups, while the u-orientation -> L0 -> W-seed
            # chain completes in their shadow
            for c in range(8):
                sl = slice(128 * c, 128 * (c + 1))
                nc.tensor.matmul(s_ps[:], EfT[:, sl], EgT[:, sl],
                                 start=(c == 0), stop=(c == 7))
            for c in range(8):
                sl = slice(128 * c, 128 * (c + 1))
                nc.tensor.matmul(s_psT[:], EgT[:, sl], EfT[:, sl],
                                 start=(c == 0), stop=(c == 7))
            nc.scalar.activation(lse_t32[:], s_ps[:], AF.Ln)
            nc.gpsimd.tensor_scalar_mul(neg_lse32[:], lse_t32[:], -1.0)
            nc.scalar.activation(lse_u[:], s_psT[:], AF.Ln)
            nc.gpsimd.tensor_scalar_mul(neg_lseT16[:], lse_u[:], -1.0)
            nc.gpsimd.tensor_scalar_mul(neg_lse16[:], lse_t32[:], -1.0)

        # flatten -lseT into partition 0 (DMA may cross partitions freely)
        # so Pool can read any u's row from an aligned window; same for the
        # leftover rows of -lse.
        L0 = const_pool.tile([1, 128 * 128], f16)
        nc.sync.dma_start(L0[0:1, :], neg_lseT16[:, :])
        L0T = const_pool.tile([1, (TSH - TMAIN) * 128], f16)
        nc.sync.dma_start(L0T[0:1, :], neg_lse16[TMAIN:TSH, :])
        # seed W lse rows (at partition 96) for u = NW..2*NW-1; the first
        # two groups (u < NW) run off the static W8 with a scalar epilogue
        for q in range(NW):
            u0 = NW + q
            nc.gpsimd.tensor_copy(
                Wb[q][96:97, :TMAIN], L0[0:1, 128 * u0:128 * u0 + TMAIN])

        # --- main loop: NGRP groups x NG u's; leftover t-rows interleaved ---
        out_pool = ctx.enter_context(tc.tile_pool(name="out", bufs=6))
        lo_pool = ctx.enter_context(tc.tile_pool(name="lo", bufs=2))
        nlo = TSH - TMAIN
        lo_every = NGRP // nlo
        with tc.tile_pool(name="psum_b", bufs=4, space="PSUM") as psum_b:
            for m in range(NGRP):
                Bb = Bt[m % NB]
                # separate stage tiles per engine: no shared writer tiles,
                # so no cross-engine serialization through the tracker
                stageD = out_pool.tile([TMAIN, 2, V], f16, tag="sD")
                stageA = out_pool.tile([TMAIN, 2, V], f16, tag="sA")
                for j in range(NG):
                    u = NG * m + j
                    hybrid = u < NW
                    Wu = (W8[:, 128 * (u % NG):128 * (u % NG) + 128]
                          if hybrid else Wb[u % NW][:, :])
                    pb = psum_b.tile([128, V], f32, tag="pb")
                    for c2 in range(2):
                        bsl = slice(512 * c2, 512 * (c2 + 1))
                        nc.tensor.matmul(
                            pb[:TMAIN, bsl], Wu[:, :TMAIN],
                            Bb[:, bsl], start=True, stop=True)
                    # refresh this W tile's lse row for u+NW (Pool,
                    # SBUF-only; WAR on the matmuls above)
                    if not hybrid and u + NW < U:
                        un = u + NW
                        nc.gpsimd.tensor_copy(
                            Wb[u % NW][96:97, :TMAIN],
                            L0[0:1, 128 * un:128 * un + TMAIN])
                    # epilogue: pure PSUM->f16 downcast; DVE takes even u's,
                    # ACT odd u's — each with its own psum and stage tile.
                    # Hybrid groups add lse here as a per-partition scalar.
                    if j % 2 == 0:
                        if hybrid:
                            nc.vector.tensor_scalar_sub(
                                stageD[:, j // 2, :], pb[:TMAIN, :],
                                lse_t32[:TMAIN, u:u + 1])
                        else:
                            nc.vector.tensor_copy(
                                stageD[:, j // 2, :], pb[:TMAIN, :])
                    else:
                        if hybrid:
                            nc.scalar.activation(
                                stageA[:, j // 2, :], pb[:TMAIN, :],
                                AF.Identity, bias=neg_lse32[:TMAIN, u:u + 1])
                        else:
                            nc.scalar.activation(
                                stageA[:, j // 2, :], pb[:TMAIN, :], AF.Copy)
                # refresh this B tile's g-rows for group m+NB
                if m + NB < NGRP:
                    nc.sync.dma_start(
                        Bt[m % NB][97:101, :],
                        G16[NG * (m + NB):NG * (m + NB + 1), :],
                    )
                u0 = NG * m
                nc.sync.dma_start(
                    out_d[0:TMAIN, u0:u0 + NG:2, :], stageD[:])
                nc.sync.dma_start(
                    out_d[0:TMAIN, u0 + 1:u0 + NG:2, :], stageA[:])

                # one leftover t-row (one-hot f_t broadcast, u-partitions;
                # lse added via a K=1 ones-row matmul)
                if m % lo_every == lo_every // 2 - 1:
                    t = TMAIN + m // lo_every
                    lse_row = L0T[0:1, 128 * (t - TMAIN):128 * (t - TMAIN + 1)]
                    pb2 = psum_b.tile([128, V], f32, tag="pb")
                    onehot = eye16[:, t:t + 1].broadcast_to([128, 128])
                    for c2 in range(2):
                        sl = slice(512 * c2, 512 * (c2 + 1))
                        nc.tensor.matmul(pb2[:, sl], onehot, F16[:, sl],
                                         start=True, stop=False)
                        nc.tensor.matmul(pb2[:, sl], eye16[:], G16[:, sl],
                                         start=False, stop=False)
                        nc.tensor.matmul(pb2[:, sl], lse_row,
                                         ones1[:, sl], start=False, stop=True)
                    stage2 = lo_pool.tile([128, V], f16)
                    nc.vector.tensor_copy(stage2[:, 0:XD], pb2[:, 0:XD])
                    nc.scalar.activation(stage2[:, XD:V], pb2[:, XD:V],
                                         AF.Copy)
                    nc.sync.dma_start(out_d[t, :, :], stage2[:])

    bacc.get_activation_tables = _steered_tables
    try:
        nc.compile()
    finally:
        bacc.get_activation_tables = _orig_tables
    _nc_cache[tag] = nc
    return nc


def _consts():
    eye16 = np.eye(128, dtype=np.float16)
    # B-row map: rows 0:96 = F[0:96], row 96 = ones (lse row in W),
    # rows 97:101 = g-rows, rows 101:128 = F[96:123]
    w8 = np.zeros((128, NG * 128), dtype=np.float16)
    for j in range(NG):
        blk = w8[:, 128 * j:128 * (j + 1)]
        for k in range(96):
            blk[k, k] = 1.0
        for k in range(101, 128):
            blk[k, k - 5] = 1.0
        for t in range(TMAIN):
            blk[97 + j, t] = 1.0
    return eye16, w8


def _in_maps(f, g):
    eye16, w8 = _consts()
    f16 = f.astype(np.float16)
    g16 = g.astype(np.float16)
    maps = []
    for i in range(NCORES):
        b, h = divmod(i, 2)
        maps.append({
            "f_sh": np.ascontiguousarray(f16[b, h * TSH:(h + 1) * TSH]),
            "g_sh": np.ascontiguousarray(g16[b]),
            "eye16": eye16,
            "w8": w8,
        })
    return maps


def _gather(results):
    out = np.empty((B, T, U, V), np.float32)
    for i in range(NCORES):
        b, h = divmod(i, 2)
        out[b, h * TSH:(h + 1) * TSH] = np.asarray(
            results[i]["out_sh"], dtype=np.float32)
    return out


def kernel(**inputs):
    from concourse.bass_utils import run_bass_kernel_spmd

    f = np.asarray(inputs["f"], np.float32)
    g = np.asarray(inputs["g"], np.float32)
    nc = _build()
    res = run_bass_kernel_spmd(nc, _in_maps(f, g), core_ids=list(range(NCORES)))
    return _gather(res.results)


# revision 38
# speedup vs baseline: 1.9165x; 1.0005x over previous
"""RNNT joint log_softmax kernel for Trainium2 (Bass/Tile), 8-core SPMD.

out[b,t,u,v] = log_softmax(f[b,t,v] + g[b,u,v], axis=v)

Sharding: 8 shards over (b, t-half): core i handles b=i//2, t in [128*(i%2), ...).
lse trick: exp(f+g) = exp(f)*exp(g), so lse[t,u] = ln(exp(f) @ exp(g)^T).

Main loop (partitions = t, loop over u): moving tile B = [F rows 0:123 |
4 g-rows | ones row], stationary W_u = diag(123) + g-select row + a row
holding -lse[:,u] (written per-u by Pool from SBUF), so ONE matmul pair per u
computes the ENTIRE output F[t,v] + g_u[v] - lse[t,u] for 123 t-rows in PSUM.
The epilogue is then a pure PSUM->fp16 downcast with no per-u scalar, so it
merges across u-pairs and splits by v-columns across DVE (tensor_copy) and
ACT (activation Copy); GPSIMD cannot touch PSUM, so Pool instead feeds the
W-row updates, B F-rows and negations (all SBUF-side). Inputs land as f16
(host-converted) and output is written fp16 (rel err ~5e-3 << 2e-2 gate),
halving HBM traffic; host upcasts. All ACT functions (Exp/Ln/Copy) are
steered into the one table set containing them so a single LoadActFuncSet
runs off the critical path. Leftover t-rows 123..127 use a one-hot broadcast
path (lse via a K=1 ones matmul), interleaved mid-loop.
"""

import numpy as np

B, T, U, V = 4, 256, 128, 1024
TSH = 128          # t-shard per core
NCORES = 8
TMAIN = 123        # t-rows handled by the B-tile trick per matmul
NG = 4             # g-rows resident in B / u's per group
NGRP = U // NG     # 32 groups
NB = 4             # B-tile ring
NW = 8             # W-stationary ring
XD = 510           # epilogue cols on DVE; rest on ACT

_nc_cache = {}


def _build(tag="main"):
    if tag in _nc_cache:
        return _nc_cache[tag]
    from contextlib import ExitStack

    import concourse.bacc as bacc
    import concourse.tile as tile
    from concourse import mybir

    f32 = mybir.dt.float32
    f16 = mybir.dt.float16
    AF = mybir.ActivationFunctionType

    # Steer Exp/Ln/Identity/Copy into the single table set holding them all
    # ("natural_log_exp_and_others") so only one LoadActFuncSet is emitted.
    # Set indices/IDs are unchanged and the chosen set genuinely contains
    # these functions, so the emitted BIR stays valid for walrus.
    _orig_tables = bacc.get_activation_tables

    def _steered_tables(arch):
        out = {}
        for name, funcs in _orig_tables(arch).items():
            funcs = set(funcs)
            if name != "natural_log_exp_and_others":
                funcs.discard(AF.Exp)
                funcs.discard(AF.Ln)
                funcs.discard(AF.Identity)
                funcs.discard(AF.Copy)
            out[name] = funcs
        return out

    nc = bacc.Bacc("TRN2", debug=False, num_devices=NCORES)
    f_d = nc.dram_tensor("f_sh", [TSH, V], f16, kind="ExternalInput").ap()
    g_d = nc.dram_tensor("g_sh", [U, V], f16, kind="ExternalInput").ap()
    eye_d = nc.dram_tensor("eye16", [128, 128], f16, kind="ExternalInput").ap()
    w8_d = nc.dram_tensor("w8", [128, NG * 128], f16, kind="ExternalInput").ap()
    ones_d = nc.dram_tensor("ones_row", [1, V], f16, kind="ExternalInput").ap()
    out_d = nc.dram_tensor("out_sh", [TSH, U, V], f16, kind="ExternalOutput").ap()

    with tile.TileContext(nc) as tc, ExitStack() as ctx:
        const_pool = ctx.enter_context(tc.tile_pool(name="const", bufs=1))

        F16 = const_pool.tile([128, V], f16)
        G16 = const_pool.tile([128, V], f16)
        eye16 = const_pool.tile([128, 128], f16)
        W8 = const_pool.tile([128, NG * 128], f16)
        scr = const_pool.tile([128, 128], f16)
        ones1 = const_pool.tile([1, V], f16)
        nc.vector.memset(scr[:], 0.0)

        Bt = [const_pool.tile([128, V], f16, name=f"Bt{q}") for q in range(NB)]

        # Input + B-seed DMAs on the SP HWDGE queue, ordered by when each
        # consumer needs the data. B ring layout (engine APs must start at
        # partition 0/32/64/96): rows 0:96 = F[0:96], row 96 = ones,
        # rows 97:101 = the group's 4 g-rows, rows 101:128 = F[96:123].
        # Seeds/tails/ones go via DMA (arbitrary partitions allowed there).
        nc.sync.dma_start(eye16[:], eye_d[:])
        nc.sync.dma_start(F16[:], f_d[:])
        nc.sync.dma_start(G16[:], g_d[:])
        nc.sync.dma_start(Bt[0][96:97, :], ones_d[:])
        nc.sync.dma_start(W8[:], w8_d[:])
        # B0 seeds early on the Pool SWDGE queue (keeps SP HWDGE short);
        # B1..3 seeds are emitted after the lse negations so Pool is free
        # when those land on its queue.
        nc.gpsimd.dma_start(Bt[0][97:101, :], g_d[0:NG, :])
        nc.gpsimd.dma_start(Bt[0][101:128, :], f_d[96:TMAIN, :])
        for q in range(1, NB):
            nc.sync.dma_start(Bt[q][96:97, :], ones_d[:])
        nc.sync.dma_start(ones1[:], ones_d[:])
        for q in range(NB):
            nc.vector.tensor_copy(Bt[q][0:96, :], F16[0:96, :])

        # --- W ring: static part from host (diag + g-select); row 127 gets
        # -lseT[u] written per-u by Pool.
        Wb = [const_pool.tile([128, 128], f16, name=f"Wb{q}") for q in range(NW)]
        for q in range(NW):
            nc.gpsimd.tensor_copy(
                Wb[q][:, :], W8[:, 128 * (q % NG):128 * (q % NG) + 128])

        # --- preamble: lse in both orientations via exp-transpose-matmul ---
        EfT = const_pool.tile([128, V], f16)   # col block c: [v-chunk, t]
        EgT = const_pool.tile([128, V], f16)
        neg_lse16 = const_pool.tile([128, 128], f16)    # [t, u]
        neg_lseT16 = const_pool.tile([128, 128], f16)   # [u, t]
        with tc.tile_pool(name="psum_pre", bufs=2, space="PSUM") as pre_psum, \
             tc.tile_pool(name="psum_s", bufs=1, space="PSUM") as s_pool:
            # PE p-state warmup while input DMAs land (results unused);
            # scr needs no DMA, so PE ramps from t~0 into the real transposes
            warm = pre_psum.tile([128, 128], f16, tag="warm")
            for _ in range(16):
                nc.tensor.transpose(warm[:], scr[:], scr[:])
            for src, dst in ((F16, EfT), (G16, EgT)):
                tp = pre_psum.tile([128, V], f16, tag="tp")
                for c in range(8):
                    nc.tensor.transpose(
                        tp[:, 128 * c:128 * (c + 1)],
                        src[:, 128 * c:128 * (c + 1)], eye16[:])
                nc.scalar.activation(dst[:], tp[:], AF.Exp)
            lse_t32 = const_pool.tile([128, 128], f32)
            neg_lse32 = const_pool.tile([128, 128], f32)
            lse_u = const_pool.tile([128, 128], f32)
            s_ps = s_pool.tile([128, 128], f32, tag="s")
            s_psT = s_pool.tile([128, 128], f32, tag="sT")
            # t-orientation first: it unblocks the hybrid scalar epilogue of
            # the first two groups, while the u-orientation -> L0 -> W-seed
            # chain completes in their shadow
            for c in range(8):
                sl = slice(128 * c, 128 * (c + 1))
                nc.tensor.matmul(s_ps[:], EfT[:, sl], EgT[:, sl],
                                 start=(c == 0), stop=(c == 7))
            for c in range(8):
                sl = slice(128 * c, 128 * (c + 1))
                nc.tensor.matmul(s_psT[:], EgT[:, sl], EfT[:, sl],
                                 start=(c == 0), stop=(c == 7))
            nc.scalar.activation(lse_t32[:, 0:32], s_ps[:, 0:32], AF.Ln)
            nc.gpsimd.tensor_scalar_mul(
                neg_lse32[:, 0:32], lse_t32[:, 0:32], -1.0)
            nc.scalar.activation(lse_t32[:, 32:], s_ps[:, 32:], AF.Ln)
            nc.gpsimd.tensor_scalar_mul(
                neg_lse32[:, 32:], lse_t32[:, 32:], -1.0)
            nc.scalar.activation(lse_u[:], s_psT[:], AF.Ln)
            nc.gpsimd.tensor_scalar_mul(neg_lseT16[:], lse_u[:], -1.0)
            nc.gpsimd.tensor_scalar_mul(neg_lse16[:], lse_t32[:], -1.0)
        for q in range(1, NB):
            nc.gpsimd.dma_start(Bt[q][97:101, :], g_d[NG * q:NG * (q + 1), :])
            nc.gpsimd.dma_start(Bt[q][101:128, :], f_d[96:TMAIN, :])

        # flatten -lseT into partition 0 (DMA may cross partitions freely)
        # so Pool can read any u's row from an aligned window; same for the
        # leftover rows of -lse.
        L0 = const_pool.tile([1, 128 * 128], f16)
        nc.sync.dma_start(L0[0:1, :], neg_lseT16[:, :])
        L0T = const_pool.tile([1, (TSH - TMAIN) * 128], f16)
        nc.sync.dma_start(L0T[0:1, :], neg_lse16[TMAIN:TSH, :])
        # seed W lse rows (at partition 96) for u = NW..2*NW-1; the first
        # two groups (u < NW) run off the static W8 with a scalar epilogue
        for q in range(NW):
            u0 = NW + q
            nc.gpsimd.tensor_copy(
                Wb[q][96:97, :TMAIN], L0[0:1, 128 * u0:128 * u0 + TMAIN])

        # --- main loop: NGRP groups x NG u's; leftover t-rows interleaved ---
        out_pool = ctx.enter_context(tc.tile_pool(name="out", bufs=6))
        lo_pool = ctx.enter_context(tc.tile_pool(name="lo", bufs=2))
        nlo = TSH - TMAIN
        lo_every = NGRP // nlo
        with tc.tile_pool(name="psum_b", bufs=4, space="PSUM") as psum_b:
            for m in range(NGRP):
                Bb = Bt[m % NB]
                # separate stage tiles per engine: no shared writer tiles,
                # so no cross-engine serialization through the tracker
                stageD = out_pool.tile([TMAIN, 2, V], f16, tag="sD")
                stageA = out_pool.tile([TMAIN, 2, V], f16, tag="sA")
                for j in range(NG):
                    u = NG * m + j
                    hybrid = u < NW
                    Wu = (W8[:, 128 * (u % NG):128 * (u % NG) + 128]
                          if hybrid else Wb[u % NW][:, :])
                    pb = psum_b.tile([128, V], f32, tag="pb")
                    for c2 in range(2):
                        bsl = slice(512 * c2, 512 * (c2 + 1))
                        nc.tensor.matmul(
                            pb[:TMAIN, bsl], Wu[:, :TMAIN],
                            Bb[:, bsl], start=True, stop=True)
                    # refresh this W tile's lse row for u+NW (Pool,
                    # SBUF-only; WAR on the matmuls above)
                    if not hybrid and u + NW < U:
                        un = u + NW
                        nc.gpsimd.tensor_copy(
                            Wb[u % NW][96:97, :TMAIN],
                            L0[0:1, 128 * un:128 * un + TMAIN])
                    # epilogue: pure PSUM->f16 downcast; DVE takes even u's,
                    # ACT odd u's — each with its own psum and stage tile.
                    # Hybrid groups add lse here as a per-partition scalar.
                    if j % 2 == 0:
                        if hybrid:
                            nc.vector.tensor_scalar_sub(
                                stageD[:, j // 2, :], pb[:TMAIN, :],
                                lse_t32[:TMAIN, u:u + 1])
                        else:
                            nc.vector.tensor_copy(
                                stageD[:, j // 2, :], pb[:TMAIN, :])
                    else:
                        if hybrid:
                            nc.scalar.activation(
                                stageA[:, j // 2, :], pb[:TMAIN, :],
                                AF.Identity, bias=neg_lse32[:TMAIN, u:u + 1])
                        else:
                            nc.scalar.activation(
                                stageA[:, j // 2, :], pb[:TMAIN, :], AF.Copy)
                # refresh this B tile's g-rows for group m+NB
                if m + NB < NGRP:
                    nc.sync.dma_start(
                        Bt[m % NB][97:101, :],
                        G16[NG * (m + NB):NG * (m + NB + 1), :],
                    )
                u0 = NG * m
                nc.sync.dma_start(
                    out_d[0:TMAIN, u0:u0 + NG:2, :], stageD[:])
                nc.sync.dma_start(
                    out_d[0:TMAIN, u0 + 1:u0 + NG:2, :], stageA[:])

                # one leftover t-row (one-hot f_t broadcast, u-partitions;
                # lse added via a K=1 ones-row matmul)
                if m % lo_every == lo_every // 2 - 1:
                    t = TMAIN + m // lo_every
                    lse_row = L0T[0:1, 128 * (t - TMAIN):128 * (t - TMAIN + 1)]
                    pb2 = psum_b.tile([128, V], f32, tag="pb")
                    onehot = eye16[:, t:t + 1].broadcast_to([128, 128])
                    for c2 in range(2):
                        sl = slice(512 * c2, 512 * (c2 + 1))
                        nc.tensor.matmul(pb2[:, sl], onehot, F16[:, sl],
                                         start=True, stop=False)
                        nc.tensor.matmul(pb2[:, sl], eye16[:], G16[:, sl],
                                         start=False, stop=False)
                        nc.tensor.matmul(pb2[:, sl], lse_row,
                                         ones1[:, sl], start=False, stop=True)
                    stage2 = lo_pool.tile([128, V], f16)
                    nc.vector.tensor_copy(stage2[:, 0:XD], pb2[:, 0:XD])
                    nc.scalar.activation(stage2[:, XD:V], pb2[:, XD:V],
                                         AF.Copy)
                    nc.sync.dma_start(out_d[t, :, :], stage2[:])

    bacc.get_activation_tables = _steered_tables
    try:
        nc.compile()
    finally:
        bacc.get_activation_tables = _orig_tables
    _nc_cache[tag] = nc
    return nc


def _consts():
    eye16 = np.eye(128, dtype=np.float16)
    # B-row map: rows 0:96 = F[0:96], row 96 = ones (lse row in W),
    # rows 97:101 = g-rows, rows 101:128 = F[96:123]
    w8 = np.zeros((128, NG * 128), dtype=np.float16)
    for j in range(NG):
        blk = w8[:, 128 * j:128 * (j + 1)]
        for k in range(96):
            blk[k, k] = 1.0
        for k in range(101, 128):
            blk[k, k - 5] = 1.0
        for t in range(TMAIN):
            blk[97 + j, t] = 1.0
    return eye16, w8


def _in_maps(f, g):
    eye16, w8 = _consts()
    ones_row = np.ones((1, V), dtype=np.float16)
    f16 = f.astype(np.float16)
    g16 = g.astype(np.float16)
    maps = []
    for i in range(NCORES):
        b, h = divmod(i, 2)
        maps.append({
            "f_sh": np.ascontiguousarray(f16[b, h * TSH:(h + 1) * TSH]),
            "g_sh": np.ascontiguousarray(g16[b]),
            "eye16": eye16,
            "w8": w8,
            "ones_row": ones_row,
        })
    return maps


def _gather(results):
    out = np.empty((B, T, U, V), np.float32)
    for i in range(NCORES):
        b, h = divmod(i, 2)
        out[b, h * TSH:(h + 1) * TSH] = np.asarray(
            results[i]["out_sh"], dtype=np.float32)
    return out


def kernel(**inputs):
    from concourse.bass_utils import run_bass_kernel_spmd

    f = np.asarray(inputs["f"], np.float32)
    g = np.asarray(inputs["g"], np.float32)
    nc = _build()
    res = run_bass_kernel_spmd(nc, _in_maps(f, g), core_ids=list(range(NCORES)))
    return _gather(res.results)


# revision 44
# speedup vs baseline: 1.9481x; 1.0165x over previous
"""RNNT joint log_softmax kernel for Trainium2 (Bass/Tile), 8-core SPMD.

out[b,t,u,v] = log_softmax(f[b,t,v] + g[b,u,v], axis=v)

Sharding: 8 shards over (b, t-half): core i handles b=i//2, t in [128*(i%2), ...).
lse trick: exp(f+g) = exp(f)*exp(g), so lse[t,u] = ln(exp(f) @ exp(g)^T).

Main loop (partitions = t, loop over u): moving tile B = [F rows | the group's
4 g-rows], constant stationary W_j = diag + g-select row, so ONE matmul pair
per u computes F[t,v] + g_u[v] for 124 t-rows in PSUM. The epilogue applies
-lse[:,u] as a per-partition scalar while downcasting PSUM->fp16; DVE
(tensor_scalar_sub) takes even u's, ACT (activation bias) odd u's — separate
psum/stage tiles per engine so the tile-granular dependency tracker never
serializes them. GPSIMD cannot touch PSUM; Pool only does SBUF-side work.
Inputs land as f16 (host-converted) and output is written fp16 (rel err
~6e-3 << 2e-2 gate), halving HBM traffic; host upcasts. All ACT functions
(Exp/Ln/Identity) are steered into the one table set containing them so a
single LoadActFuncSet runs off the critical path. Engine APs must start at
partition 0/32/64/96, so the 4 g-rows sit at 96:100 (written by DMA, which
has no such limit) and leftover t-rows 124..127 go through a one-hot
broadcast path (lse added via a K=1 ones-row matmul), interleaved mid-loop.
"""

import numpy as np

B, T, U, V = 4, 256, 128, 1024
TSH = 128          # t-shard per core
NCORES = 8
TMAIN = 124        # t-rows handled by the B-tile trick per matmul
NG = 4             # g-rows resident in B / u's per group
NGRP = U // NG     # 32 groups
NB = 4             # B-tile ring

_nc_cache = {}


def _build(tag="main"):
    if tag in _nc_cache:
        return _nc_cache[tag]
    from contextlib import ExitStack

    import concourse.bacc as bacc
    import concourse.tile as tile
    from concourse import mybir

    f32 = mybir.dt.float32
    f16 = mybir.dt.float16
    AF = mybir.ActivationFunctionType

    # Steer Exp/Ln/Identity/Copy into the single table set holding them all
    # ("natural_log_exp_and_others") so only one LoadActFuncSet is emitted.
    # Set indices/IDs are unchanged and the chosen set genuinely contains
    # these functions, so the emitted BIR stays valid for walrus.
    _orig_tables = bacc.get_activation_tables

    def _steered_tables(arch):
        out = {}
        for name, funcs in _orig_tables(arch).items():
            funcs = set(funcs)
            if name != "natural_log_exp_and_others":
                funcs.discard(AF.Exp)
                funcs.discard(AF.Ln)
                funcs.discard(AF.Identity)
                funcs.discard(AF.Copy)
            out[name] = funcs
        return out

    nc = bacc.Bacc("TRN2", debug=False, num_devices=NCORES)
    f_d = nc.dram_tensor("f_sh", [TSH, V], f16, kind="ExternalInput").ap()
    g_d = nc.dram_tensor("g_sh", [U, V], f16, kind="ExternalInput").ap()
    eye_d = nc.dram_tensor("eye16", [128, 128], f16, kind="ExternalInput").ap()
    w8_d = nc.dram_tensor("w8", [128, NG * 128], f16, kind="ExternalInput").ap()
    ones_d = nc.dram_tensor("ones_row", [1, V], f16, kind="ExternalInput").ap()
    out_d = nc.dram_tensor("out_sh", [TSH, U, V], f16, kind="ExternalOutput").ap()

    with tile.TileContext(nc) as tc, ExitStack() as ctx:
        const_pool = ctx.enter_context(tc.tile_pool(name="const", bufs=1))

        F16 = const_pool.tile([128, V], f16)
        G16 = const_pool.tile([128, V], f16)
        eye16 = const_pool.tile([128, 128], f16)
        W8 = const_pool.tile([128, NG * 128], f16)
        scr = const_pool.tile([128, 128], f16)
        ones1 = const_pool.tile([1, V], f16)
        nc.vector.memset(scr[:], 0.0)

        # B ring layout (engine APs must start at partition 0/32/64/96):
        # rows 0:96 = F[0:96], rows 96:100 = the group's 4 g-rows,
        # rows 100:128 = F[96:124]. Seeds/tails go via DMA (no partition
        # limits there); B0's seeds ride the Pool SWDGE queue to keep the
        # SP HWDGE queue short early on.
        Bt = [const_pool.tile([128, V], f16, name=f"Bt{q}") for q in range(NB)]
        nc.sync.dma_start(eye16[:], eye_d[:])
        nc.sync.dma_start(F16[:], f_d[:])
        nc.sync.dma_start(G16[:], g_d[:])
        nc.sync.dma_start(W8[:], w8_d[:])
        nc.sync.dma_start(ones1[:], ones_d[:])
        nc.gpsimd.dma_start(Bt[0][96:100, :], g_d[0:NG, :])
        nc.gpsimd.dma_start(Bt[0][100:128, :], f_d[96:TMAIN, :])
        for q in range(NB):
            nc.vector.tensor_copy(Bt[q][0:96, :], F16[0:96, :])

        # --- preamble: lse (t-orientation) via exp-transpose-matmul ---
        EfT = const_pool.tile([128, V], f16)   # col block c: [v-chunk, t]
        EgT = const_pool.tile([128, V], f16)
        neg_lse16 = const_pool.tile([128, 128], f16)    # [t, u] (leftovers)
        with tc.tile_pool(name="psum_pre", bufs=2, space="PSUM") as pre_psum, \
             tc.tile_pool(name="psum_s", bufs=1, space="PSUM") as s_pool:
            # PE p-state warmup while input DMAs land (results unused);
            # scr needs no DMA, so PE ramps from t~0 into the real transposes
            warm = pre_psum.tile([128, 128], f16, tag="warm")
            for _ in range(16):
                nc.tensor.transpose(warm[:], scr[:], scr[:])
            for src, dst in ((F16, EfT), (G16, EgT)):
                tp = pre_psum.tile([128, V], f16, tag="tp")
                for c in range(8):
                    nc.tensor.transpose(
                        tp[:, 128 * c:128 * (c + 1)],
                        src[:, 128 * c:128 * (c + 1)], eye16[:])
                nc.scalar.activation(dst[:], tp[:], AF.Exp)
            lse_t32 = const_pool.tile([128, 128], f32)
            neg_lse32 = const_pool.tile([128, 128], f32)
            s_ps = s_pool.tile([128, 128], f32, tag="s")
            for c in range(8):
                sl = slice(128 * c, 128 * (c + 1))
                nc.tensor.matmul(s_ps[:], EfT[:, sl], EgT[:, sl],
                                 start=(c == 0), stop=(c == 7))
            # u-slice the Ln/negate so the first groups unblock earliest
            nc.scalar.activation(lse_t32[:, 0:32], s_ps[:, 0:32], AF.Ln)
            nc.gpsimd.tensor_scalar_mul(
                neg_lse32[:, 0:32], lse_t32[:, 0:32], -1.0)
            nc.scalar.activation(lse_t32[:, 32:], s_ps[:, 32:], AF.Ln)
            nc.gpsimd.tensor_scalar_mul(
                neg_lse32[:, 32:], lse_t32[:, 32:], -1.0)
            nc.gpsimd.tensor_scalar_mul(neg_lse16[:], lse_t32[:], -1.0)
        # remaining B seeds once Pool's queue is clear
        for q in range(1, NB):
            nc.gpsimd.dma_start(Bt[q][96:100, :], g_d[NG * q:NG * (q + 1), :])
            nc.gpsimd.dma_start(Bt[q][100:128, :], f_d[96:TMAIN, :])

        # leftover rows' -lse flattened into partition 0 (DMA may cross
        # partitions freely) for the K=1 ones-row matmul stationary
        L0T = const_pool.tile([1, (TSH - TMAIN) * 128], f16)
        nc.sync.dma_start(L0T[0:1, :], neg_lse16[TMAIN:TSH, :])

        # --- main loop: NGRP groups x NG u's; leftover t-rows interleaved ---
        out_pool = ctx.enter_context(tc.tile_pool(name="out", bufs=6))
        lo_pool = ctx.enter_context(tc.tile_pool(name="lo", bufs=2))
        nlo = TSH - TMAIN
        lo_every = NGRP // nlo
        with tc.tile_pool(name="psum_b", bufs=4, space="PSUM") as psum_b:
            for m in range(NGRP):
                Bb = Bt[m % NB]
                # separate stage tiles per engine: no shared writer tiles,
                # so no cross-engine serialization through the tracker
                stageD = out_pool.tile([TMAIN, 2, V], f16, tag="sD")
                stageA = out_pool.tile([TMAIN, 2, V], f16, tag="sA")
                for j in range(NG):
                    u = NG * m + j
                    Wj = W8[:, 128 * j:128 * j + TMAIN]
                    pb = psum_b.tile([128, V], f32, tag="pb")
                    for c2 in range(2):
                        bsl = slice(512 * c2, 512 * (c2 + 1))
                        nc.tensor.matmul(
                            pb[:TMAIN, bsl], Wj, Bb[:, bsl],
                            start=True, stop=True)
                    # epilogue: PSUM->f16 downcast with -lse[:,u] applied as
                    # a per-partition scalar; DVE takes even u's, ACT odd
                    if j % 2 == 0:
                        nc.vector.tensor_scalar_sub(
                            stageD[:, j // 2, :], pb[:TMAIN, :],
                            lse_t32[:TMAIN, u:u + 1])
                    else:
                        nc.scalar.activation(
                            stageA[:, j // 2, :], pb[:TMAIN, :],
                            AF.Identity, bias=neg_lse32[:TMAIN, u:u + 1])
                # refresh this B tile's g-rows for group m+NB
                if m + NB < NGRP:
                    nc.sync.dma_start(
                        Bt[m % NB][96:100, :],
                        G16[NG * (m + NB):NG * (m + NB + 1), :],
                    )
                u0 = NG * m
                nc.sync.dma_start(
                    out_d[0:TMAIN, u0:u0 + NG:2, :], stageD[:])
                nc.sync.dma_start(
                    out_d[0:TMAIN, u0 + 1:u0 + NG:2, :], stageA[:])

                # one leftover t-row (one-hot f_t broadcast, u-partitions;
                # lse added via a K=1 ones-row matmul), interleaved
                if m % lo_every == lo_every // 2 - 1:
                    t = TMAIN + m // lo_every
                    lse_row = L0T[0:1, 128 * (t - TMAIN):128 * (t - TMAIN + 1)]
                    pb2 = psum_b.tile([128, V], f32, tag="pb")
                    onehot = eye16[:, t:t + 1].broadcast_to([128, 128])
                    for c2 in range(2):
                        sl = slice(512 * c2, 512 * (c2 + 1))
                        nc.tensor.matmul(pb2[:, sl], onehot, F16[:, sl],
                                         start=True, stop=False)
                        nc.tensor.matmul(pb2[:, sl], eye16[:], G16[:, sl],
                                         start=False, stop=False)
                        nc.tensor.matmul(pb2[:, sl], lse_row,
                                         ones1[:, sl], start=False, stop=True)
                    stage2 = lo_pool.tile([128, V], f16)
                    nc.vector.tensor_copy(stage2[:, 0:512], pb2[:, 0:512])
                    nc.scalar.activation(stage2[:, 512:], pb2[:, 512:],
                                         AF.Copy)
                    nc.sync.dma_start(out_d[t, :, :], stage2[:])

    bacc.get_activation_tables = _steered_tables
    try:
        nc.compile()
    finally:
        bacc.get_activation_tables = _orig_tables
    _nc_cache[tag] = nc
    return nc


def _consts():
    eye16 = np.eye(128, dtype=np.float16)
    # B-row map: rows 0:96 = F[0:96], rows 96:100 = g-rows,
    # rows 100:128 = F[96:124]
    w8 = np.zeros((128, NG * 128), dtype=np.float16)
    for j in range(NG):
        blk = w8[:, 128 * j:128 * (j + 1)]
        for k in range(96):
            blk[k, k] = 1.0
        for k in range(100, 128):
            blk[k, k - 4] = 1.0
        for t in range(TMAIN):
            blk[96 + j, t] = 1.0
    return eye16, w8


def _in_maps(f, g):
    eye16, w8 = _consts()
    ones_row = np.ones((1, V), dtype=np.float16)
    f16 = f.astype(np.float16)
    g16 = g.astype(np.float16)
    maps = []
    for i in range(NCORES):
        b, h = divmod(i, 2)
        maps.append({
            "f_sh": np.ascontiguousarray(f16[b, h * TSH:(h + 1) * TSH]),
            "g_sh": np.ascontiguousarray(g16[b]),
            "eye16": eye16,
            "w8": w8,
            "ones_row": ones_row,
        })
    return maps


def _gather(results):
    out = np.empty((B, T, U, V), np.float32)
    for i in range(NCORES):
        b, h = divmod(i, 2)
        out[b, h * TSH:(h + 1) * TSH] = np.asarray(
            results[i]["out_sh"], dtype=np.float32)
    return out


def kernel(**inputs):
    from concourse.bass_utils import run_bass_kernel_spmd

    f = np.asarray(inputs["f"], np.float32)
    g = np.asarray(inputs["g"], np.float32)
    nc = _build()
    res = run_bass_kernel_spmd(nc, _in_maps(f, g), core_ids=list(range(NCORES)))
    return _gather(res.results)
